# revision 41
# baseline (speedup 1.0000x reference)
"""CLIP contrastive loss on 8 Trainium2 NeuronCores (Bass/Tile).

Strategy (data-parallel over image rows, hint's local_loss path):
  - Core c holds image rows [c*1024, (c+1)*1024) and the FULL text matrix.
  - Text rows are rolled by c*1024 on the host so the compiled program is
    core-independent.
  - Inputs are quantized to fp8 (e4m3) on the host; the 1024x8192 logits
    block is computed with DoubleRow fp8 matmuls (K=256 per instruction,
    2x bf16 PE throughput). Loss impact of fp8 inputs measured at
    rel-err ~5e-4, far inside the 2e-2 gate.
  - exp(scale*s - shift) of each 2048-wide PSUM stripe tile (2 in
    flight) runs on ScalarE with accum_out giving the per-row sums for
    free; ScalarE's back-to-back exp cadence (~2.07us/tile) paces the
    steady state with zero pipeline bubbles.
  - Column sums accumulate on VectorE into two per-stripe [128, 2048]
    bf16 accumulators (even/odd m-tiles, summed on host) so the even
    chain finishes a tile early and its writeback overlaps the drain.
  - Input DMAs are spread across the three DMA-capable queues
    (sync / scalar / gpsimd); stripe 0 is split by column half across
    scalar+gpsimd so the first exp starts ~1.5us sooner.
  - Warm-up matmuls on zeroed SBUF run during the ~7us framework
    preamble so the PE's HAM clock gate is already released (2.4 GHz)
    when the first real matmul issues.
  - The very last m-tile is split 1536/512 so the final exp/accumulate/
    writeback drain is short.
  - Host: diagonal computed exactly from the fp32 inputs, per-core
    row/col exp-sums combined in float64:
      lse = shift + log(sum); loss = mean over both directions.

Fixed-shift logsumexp is numerically safe: logits = scale*cos(theta)
are bounded by +-scale, and shift = scale/2 keeps every term that
matters in normal f32/bf16 range.
"""

from contextlib import ExitStack

import numpy as np

import concourse.bass as bass
from concourse import bacc
import concourse.tile as tile
from concourse import mybir
from concourse.bass import ts
from concourse.bass_utils import run_bass_kernel_spmd

N = 8192
D = 512
NC = 8
M_LOC = N // NC          # 1024 image rows per core
MT = M_LOC // 128        # 8 m-tiles of 128 rows
KC = D // 128            # 4 contraction chunks of 128 (paired 2x for DoubleRow)
SW = 2048                # stripe width (one PSUM tile = 4 banks)
NT = N // SW             # 4 stripes

F32 = mybir.dt.float32
BF16 = mybir.dt.bfloat16
FP8 = mybir.dt.float8e4

MM_DTYPE = "fp8dr"       # informational (test.py prints it)

_CACHE = {}
LAST_RESULTS = None


def _build(scale: float, shift: float):
    nc = bacc.Bacc("TRN2", debug=False)

    at_d = nc.dram_tensor("at_in", [128, KC, M_LOC], FP8, kind="ExternalInput").ap()
    bt_d = nc.dram_tensor("bt_in", [128, KC, N], FP8, kind="ExternalInput").ap()

    # last slot (index MT) holds the second half-tile row sums of the
    # split final m-tile; host adds it into row mt=7 of stripe NT-1.
    rowp_d = nc.dram_tensor("rowpart_out", [128, MT + 1, NT], F32, kind="ExternalOutput").ap()
    # [nt, 0] = even-mt accumulator, [nt, 1] = odd-mt accumulator (host sums)
    colsum_d = nc.dram_tensor("colsum_out", [NT, 2, 128, SW], BF16, kind="ExternalOutput").ap()

    with ExitStack() as ctx:
        tc = ctx.enter_context(tile.TileContext(nc))
        singles = ctx.enter_context(tc.tile_pool(name="singles", bufs=1))
        colp = ctx.enter_context(tc.tile_pool(name="colp", bufs=6))
        expp = ctx.enter_context(tc.tile_pool(name="expp", bufs=5))
        psum = ctx.enter_context(tc.tile_pool(name="psum", bufs=2, space="PSUM"))

        at_t = singles.tile([128, KC, M_LOC], FP8)
        bt_t = singles.tile([128, KC, N], FP8)
        bias_t = singles.tile([128, 1], F32)
        nc.vector.memset(bias_t, -shift)
        rowpart = singles.tile([128, MT + 1, NT], F32)
        warm_in = singles.tile([128, 2, 512], FP8)
        nc.vector.memset(warm_in, 0.0)

        # Input DMAs. sync: at in two chunks (first m-tiles first), then
        # the outputs. Stripe 0 is split by column half across the scalar
        # and gpsimd queues so its last chunk (which gates the first exp)
        # lands as early as possible; later stripes alternate queues and
        # all arrive well ahead of the compute.
        nc.sync.dma_start(at_t[:, :, 0:256], at_d[:, :, 0:256])
        nc.sync.dma_start(at_t[:, :, 256:1024], at_d[:, :, 256:1024])
        nc.scalar.dma_start(bt_t[:, 0:2, 0:1024], bt_d[:, 0:2, 0:1024])
        nc.gpsimd.dma_start(bt_t[:, 0:2, 1024:2048], bt_d[:, 0:2, 1024:2048])
        nc.scalar.dma_start(bt_t[:, 2:4, 0:1024], bt_d[:, 2:4, 0:1024])
        nc.gpsimd.dma_start(bt_t[:, 2:4, 1024:2048], bt_d[:, 2:4, 1024:2048])
        nc.scalar.dma_start(bt_t[:, 0:2, ts(1, SW)], bt_d[:, 0:2, ts(1, SW)])
        nc.scalar.dma_start(bt_t[:, 2:4, ts(1, SW)], bt_d[:, 2:4, ts(1, SW)])
        for nt in (2, 3):
            nc.gpsimd.dma_start(bt_t[:, 0:2, ts(nt, SW)], bt_d[:, 0:2, ts(nt, SW)])
            nc.gpsimd.dma_start(bt_t[:, 2:4, ts(nt, SW)], bt_d[:, 2:4, ts(nt, SW)])

        # Warm-up matmuls on zeroed SBUF (no DMA dependency): they run
        # during the preamble + input-transfer window.
        warm_ps = psum.tile([128, SW], F32, name="warm", tag="spsum")
        for w in range(10):
            nc.tensor.matmul(
                warm_ps[:, 0:512],
                warm_in[:, :, 0:128],
                warm_in,
                start=True,
                stop=True,
                perf_mode=mybir.MatmulPerfMode.DoubleRow,
            )

        for nt in range(NT):
            cacc_a = colp.tile([128, SW], BF16, name=f"cacca{nt}", tag="cacc")
            cacc_b = colp.tile([128, SW], BF16, name=f"caccb{nt}", tag="cacc")
            for mt in range(MT):
                s_ps = psum.tile([128, SW], F32, name=f"s{nt}_{mt}", tag="spsum")
                for i in range(2):  # K=256 DoubleRow chunks
                    for h in range(SW // 512):
                        nc.tensor.matmul(
                            s_ps[:, ts(h, 512)],
                            at_t[:, 2 * i : 2 * i + 2, ts(mt, 128)],
                            bt_t[:, 2 * i : 2 * i + 2, nt * SW + h * 512 : nt * SW + (h + 1) * 512],
                            start=(i == 0),
                            stop=(i == 1),
                            perf_mode=mybir.MatmulPerfMode.DoubleRow,
                        )
                # The last stripe accumulates into a SINGLE chain: half the
                # bytes to write back on the drain path.
                last_stripe = nt == NT - 1
                n_init = 1 if last_stripe else 2
                cacc = cacc_a if (last_stripe or mt % 2 == 0) else cacc_b
                split = last_stripe and mt == MT - 1
                e_t = (
                    cacc
                    if mt < n_init
                    else expp.tile([128, SW], BF16, name=f"e{nt}_{mt}", tag="exp")
                )
                halves = ((0, SW, mt),) if not split else ((0, 1536, mt), (1536, SW, MT))
                for lo, hi, slot in halves:
                    nc.scalar.activation(
                        e_t[:, lo:hi],
                        s_ps[:, lo:hi],
                        mybir.ActivationFunctionType.Exp,
                        bias=bias_t,
                        scale=scale,
                        accum_out=rowpart[:, slot, nt : nt + 1],
                    )
                    if mt >= n_init:
                        nc.vector.tensor_add(cacc[:, lo:hi], cacc[:, lo:hi], e_t[:, lo:hi])
            # A finishes one tile before B: write it back immediately. The
            # final stripe's writebacks are on the drain path, so they go
            # out in pieces spread over the three queues as soon as each
            # piece's accumulate finishes.
            if nt == NT - 1:
                # single chain: 0.5 MB total, first two chunks leave after
                # the 1536-half's accumulate, the 512 tail goes last.
                nc.sync.dma_start(colsum_d[nt, 0, :, 0:768], cacc_a[:, 0:768])
                nc.scalar.dma_start(colsum_d[nt, 0, :, 768:1536], cacc_a[:, 768:1536])
                nc.gpsimd.dma_start(colsum_d[nt, 0, :, 1536:SW], cacc_a[:, 1536:SW])
                nc.sync.dma_start(rowp_d, rowpart)
            else:
                nc.sync.dma_start(colsum_d[nt, 0], cacc_a)
                nc.sync.dma_start(colsum_d[nt, 1], cacc_b)

    nc.compile()
    return nc


def _prep_inputs(img, txt):
    import ml_dtypes

    fp8 = ml_dtypes.float8_e4m3
    img8 = img.astype(fp8)
    txt8 = txt.astype(fp8)
    in_maps = []
    for c in range(NC):
        A8 = img8[c * M_LOC : (c + 1) * M_LOC]                  # [1024, 512]
        at = np.ascontiguousarray(
            A8.T.reshape(KC, 128, M_LOC).transpose(1, 0, 2)
        )                                                       # [128, 4, 1024]
        tr8 = np.roll(txt8, -c * M_LOC, axis=0)                 # local col j -> global (j + c*1024) % N
        bt = np.ascontiguousarray(
            tr8.T.reshape(KC, 128, N).transpose(1, 0, 2)
        )                                                       # [128, 4, 8192]
        in_maps.append({"at_in": at, "bt_in": bt})
    return in_maps


def kernel(image_features, text_features, logit_scale):
    global LAST_RESULTS
    img = np.ascontiguousarray(np.asarray(image_features, dtype=np.float32))
    txt = np.ascontiguousarray(np.asarray(text_features, dtype=np.float32))
    scale = float(np.asarray(logit_scale))
    shift = 0.5 * scale

    key = (scale,)
    if key not in _CACHE:
        _CACHE[key] = _build(scale, shift)
    nc = _CACHE[key]

    in_maps = _prep_inputs(img, txt)
    res = run_bass_kernel_spmd(nc, in_maps, core_ids=list(range(NC)))
    LAST_RESULTS = res

    # exact diagonal from the fp32 inputs
    diag = scale * np.einsum("ij,ij->i", img.astype(np.float64), txt.astype(np.float64))

    colsum_tot = np.zeros(N, dtype=np.float64)
    lse_rows = []
    for c, r in enumerate(res.results):
        rp = r["rowpart_out"].astype(np.float64)                  # [128, MT+1, NT]
        rp[:, MT - 1, NT - 1] += rp[:, MT, NT - 1]                # fold split half-tile
        rowsum = rp[:, :MT, :].sum(axis=2)                        # [128, MT]
        lse_rows.append(shift + np.log(rowsum.T.reshape(-1)))     # row = mt*128 + p
        cs = r["colsum_out"].astype(np.float64)                   # [NT, 2, 128, SW]
        colsum_local = cs[:, 0] + cs[:, 1]                        # [NT, 128, SW]
        colsum_local[NT - 1] = cs[NT - 1, 0]                      # last stripe: single chain
        colsum_tot += np.roll(colsum_local.sum(axis=1).reshape(-1), c * M_LOC)
    lse_row = np.concatenate(lse_rows)
    lse_col = shift + np.log(colsum_tot)

    loss = 0.5 * (np.mean(lse_row - diag) + np.mean(lse_col - diag))
    return np.float32(loss)


# revision 45
# speedup vs baseline: 1.1467x; 1.1467x over previous
"""CLIP contrastive loss on 8 Trainium2 NeuronCores (Bass/Tile).

Strategy (data-parallel over image rows, hint's local_loss path):
  - Core c holds image rows [c*1024, (c+1)*1024) and the FULL text matrix.
  - Text rows are rolled by c*1024 on the host so the compiled program is
    core-independent.
  - Inputs are quantized to fp8 (e4m3) on the host; the 1024x8192 logits
    block is computed with DoubleRow fp8 matmuls (K=256 per instruction,
    2x bf16 PE throughput). Loss impact of fp8 inputs measured at
    rel-err ~5e-4, far inside the 2e-2 gate.
  - exp(scale*s - shift) of each 2048-wide PSUM stripe tile (2 in
    flight) runs on ScalarE with accum_out giving the per-row sums for
    free; ScalarE's back-to-back exp cadence (~2.07us/tile) paces the
    steady state with zero pipeline bubbles.
  - Column sums accumulate on VectorE into two per-stripe [128, 2048]
    bf16 accumulators (even/odd m-tiles, summed on host) so the even
    chain finishes a tile early and its writeback overlaps the drain.
  - Input DMAs are spread across the three DMA-capable queues
    (sync / scalar / gpsimd); stripe 0 is split by column half across
    scalar+gpsimd so the first exp starts ~1.5us sooner.
  - Warm-up matmuls on zeroed SBUF run during the ~7us framework
    preamble so the PE's HAM clock gate is already released (2.4 GHz)
    when the first real matmul issues.
  - The very last m-tile is split 1536/512 so the final exp/accumulate/
    writeback drain is short.
  - Host: diagonal computed exactly from the fp32 inputs, per-core
    row/col exp-sums combined in float64:
      lse = shift + log(sum); loss = mean over both directions.

Fixed-shift logsumexp is numerically safe: logits = scale*cos(theta)
are bounded by +-scale, and shift = scale/2 keeps every term that
matters in normal f32/bf16 range.
"""

from contextlib import ExitStack

import numpy as np

import concourse.bass as bass
from concourse import bacc
import concourse.tile as tile
from concourse import mybir
from concourse.bass import ts
from concourse.bass_utils import run_bass_kernel_spmd

N = 8192
D = 512
NC = 8
M_LOC = N // NC          # 1024 image rows per core
MT = M_LOC // 128        # 8 m-tiles of 128 rows
KC = D // 128            # 4 contraction chunks of 128 (paired 2x for DoubleRow)
SW = 2048                # stripe width (one PSUM tile = 4 banks)
NT = N // SW             # 4 stripes

F32 = mybir.dt.float32
BF16 = mybir.dt.bfloat16
FP8 = mybir.dt.float8e4

MM_DTYPE = "fp8dr"       # informational (test.py prints it)

_CACHE = {}
LAST_RESULTS = None


def _build(scale: float, shift: float):
    nc = bacc.Bacc("TRN2", debug=False)

    at_d = nc.dram_tensor("at_in", [128, KC, M_LOC], FP8, kind="ExternalInput").ap()
    bt_d = nc.dram_tensor("bt_in", [128, KC, N], FP8, kind="ExternalInput").ap()

    # last slot (index MT) holds the second half-tile row sums of the
    # split final m-tile; host adds it into row mt=7 of stripe NT-1.
    rowp_d = nc.dram_tensor("rowpart_out", [128, MT + 1, NT], F32, kind="ExternalOutput").ap()
    # [nt, 0] = even-mt accumulator, [nt, 1] = odd-mt accumulator (host sums)
    colsum_d = nc.dram_tensor("colsum_out", [NT, 2, 128, SW], BF16, kind="ExternalOutput").ap()

    with ExitStack() as ctx:
        tc = ctx.enter_context(tile.TileContext(nc))
        singles = ctx.enter_context(tc.tile_pool(name="singles", bufs=1))
        colp = ctx.enter_context(tc.tile_pool(name="colp", bufs=6))
        expp = ctx.enter_context(tc.tile_pool(name="expp", bufs=5))
        psum = ctx.enter_context(tc.tile_pool(name="psum", bufs=2, space="PSUM"))

        at_t = singles.tile([128, KC, M_LOC], FP8)
        bt_t = singles.tile([128, KC, N], FP8)
        bias_t = singles.tile([128, 1], F32)
        nc.vector.memset(bias_t, -shift)
        rowpart = singles.tile([128, MT + 1, NT], F32)
        warm_in = singles.tile([128, 2, 512], FP8)
        nc.vector.memset(warm_in, 0.0)

        # Input DMAs. sync: at in two chunks (first m-tiles first), then
        # the outputs. Stripe 0 is split by column half across the scalar
        # and gpsimd queues so its last chunk (which gates the first exp)
        # lands as early as possible; later stripes alternate queues and
        # all arrive well ahead of the compute.
        nc.sync.dma_start(at_t[:, :, 0:256], at_d[:, :, 0:256])
        nc.sync.dma_start(at_t[:, :, 256:1024], at_d[:, :, 256:1024])
        nc.scalar.dma_start(bt_t[:, 0:2, 0:1024], bt_d[:, 0:2, 0:1024])
        nc.gpsimd.dma_start(bt_t[:, 0:2, 1024:2048], bt_d[:, 0:2, 1024:2048])
        nc.scalar.dma_start(bt_t[:, 2:4, 0:1024], bt_d[:, 2:4, 0:1024])
        nc.gpsimd.dma_start(bt_t[:, 2:4, 1024:2048], bt_d[:, 2:4, 1024:2048])
        nc.scalar.dma_start(bt_t[:, 0:2, ts(1, SW)], bt_d[:, 0:2, ts(1, SW)])
        nc.scalar.dma_start(bt_t[:, 2:4, ts(1, SW)], bt_d[:, 2:4, ts(1, SW)])
        for nt in (2, 3):
            nc.gpsimd.dma_start(bt_t[:, 0:2, ts(nt, SW)], bt_d[:, 0:2, ts(nt, SW)])
            nc.gpsimd.dma_start(bt_t[:, 2:4, ts(nt, SW)], bt_d[:, 2:4, ts(nt, SW)])

        # Warm-up matmuls on zeroed SBUF (no DMA dependency): they run
        # during the preamble + input-transfer window.
        warm_ps = psum.tile([128, SW], F32, name="warm", tag="spsum")
        for w in range(10):
            nc.tensor.matmul(
                warm_ps[:, 0:512],
                warm_in[:, :, 0:128],
                warm_in,
                start=True,
                stop=True,
                perf_mode=mybir.MatmulPerfMode.DoubleRow,
            )

        for nt in range(NT):
            cacc_a = colp.tile([128, SW], BF16, name=f"cacca{nt}", tag="cacc")
            cacc_b = colp.tile([128, SW], BF16, name=f"caccb{nt}", tag="cacc")
            for mt in range(MT):
                s_ps = psum.tile([128, SW], F32, name=f"s{nt}_{mt}", tag="spsum")
                for i in range(2):  # K=256 DoubleRow chunks
                    for h in range(SW // 512):
                        nc.tensor.matmul(
                            s_ps[:, ts(h, 512)],
                            at_t[:, 2 * i : 2 * i + 2, ts(mt, 128)],
                            bt_t[:, 2 * i : 2 * i + 2, nt * SW + h * 512 : nt * SW + (h + 1) * 512],
                            start=(i == 0),
                            stop=(i == 1),
                            perf_mode=mybir.MatmulPerfMode.DoubleRow,
                        )
                cacc = cacc_a if mt % 2 == 0 else cacc_b
                split = nt == NT - 1 and mt == MT - 1
                e_t = (
                    cacc
                    if mt < 2
                    else expp.tile([128, SW], BF16, name=f"e{nt}_{mt}", tag="exp")
                )
                halves = ((0, SW, mt),) if not split else ((0, 1536, mt), (1536, SW, MT))
                for lo, hi, slot in halves:
                    nc.scalar.activation(
                        e_t[:, lo:hi],
                        s_ps[:, lo:hi],
                        mybir.ActivationFunctionType.Exp,
                        bias=bias_t,
                        scale=scale,
                        accum_out=rowpart[:, slot, nt : nt + 1],
                    )
                    if mt >= 2:
                        nc.vector.tensor_add(cacc[:, lo:hi], cacc[:, lo:hi], e_t[:, lo:hi])
            # A finishes one tile before B: write it back immediately. The
            # final stripe's writebacks are on the drain path, so they go
            # out in pieces spread over the three queues as soon as each
            # piece's accumulate finishes.
            if nt == NT - 1:
                nc.sync.dma_start(colsum_d[nt, 0, :, 0:1024], cacc_a[:, 0:1024])
                nc.gpsimd.dma_start(colsum_d[nt, 0, :, 1024:SW], cacc_a[:, 1024:SW])
                # B leaves in three pieces: [0:1536] unblocks at the
                # 1536-half's accumulate, only the 512 tail waits for the end
                nc.scalar.dma_start(colsum_d[nt, 1, :, 0:1024], cacc_b[:, 0:1024])
                nc.sync.dma_start(colsum_d[nt, 1, :, 1024:1536], cacc_b[:, 1024:1536])
                nc.gpsimd.dma_start(colsum_d[nt, 1, :, 1536:SW], cacc_b[:, 1536:SW])
                nc.sync.dma_start(rowp_d, rowpart)
            else:
                nc.sync.dma_start(colsum_d[nt, 0], cacc_a)
                nc.sync.dma_start(colsum_d[nt, 1], cacc_b)

    nc.compile()
    return nc


def _prep_inputs(img, txt):
    import ml_dtypes

    fp8 = ml_dtypes.float8_e4m3
    img8 = img.astype(fp8)
    txt8 = txt.astype(fp8)
    in_maps = []
    for c in range(NC):
        A8 = img8[c * M_LOC : (c + 1) * M_LOC]                  # [1024, 512]
        at = np.ascontiguousarray(
            A8.T.reshape(KC, 128, M_LOC).transpose(1, 0, 2)
        )                                                       # [128, 4, 1024]
        tr8 = np.roll(txt8, -c * M_LOC, axis=0)                 # local col j -> global (j + c*1024) % N
        bt = np.ascontiguousarray(
            tr8.T.reshape(KC, 128, N).transpose(1, 0, 2)
        )                                                       # [128, 4, 8192]
        in_maps.append({"at_in": at, "bt_in": bt})
    return in_maps


def kernel(image_features, text_features, logit_scale):
    global LAST_RESULTS
    img = np.ascontiguousarray(np.asarray(image_features, dtype=np.float32))
    txt = np.ascontiguousarray(np.asarray(text_features, dtype=np.float32))
    scale = float(np.asarray(logit_scale))
    shift = 0.5 * scale

    key = (scale,)
    if key not in _CACHE:
        _CACHE[key] = _build(scale, shift)
    nc = _CACHE[key]

    in_maps = _prep_inputs(img, txt)
    res = run_bass_kernel_spmd(nc, in_maps, core_ids=list(range(NC)))
    LAST_RESULTS = res

    # exact diagonal from the fp32 inputs
    diag = scale * np.einsum("ij,ij->i", img.astype(np.float64), txt.astype(np.float64))

    colsum_tot = np.zeros(N, dtype=np.float64)
    lse_rows = []
    for c, r in enumerate(res.results):
        rp = r["rowpart_out"].astype(np.float64)                  # [128, MT+1, NT]
        rp[:, MT - 1, NT - 1] += rp[:, MT, NT - 1]                # fold split half-tile
        rowsum = rp[:, :MT, :].sum(axis=2)                        # [128, MT]
        lse_rows.append(shift + np.log(rowsum.T.reshape(-1)))     # row = mt*128 + p
        colsum_tot += np.roll(
            r["colsum_out"].astype(np.float64).sum(axis=(1, 2)).reshape(-1), c * M_LOC
        )
    lse_row = np.concatenate(lse_rows)
    lse_col = shift + np.log(colsum_tot)

    loss = 0.5 * (np.mean(lse_row - diag) + np.mean(lse_col - diag))
    return np.float32(loss)
